# revision 36
# baseline (speedup 1.0000x reference)
"""Multi-head attention TRN2 kernel, sharded over 8 NeuronCores.

Sharding: (batch, head-group) - core c handles batch c//4 and heads
(c%4)*4 .. (c%4)*4+3. Each core computes its 4 heads' attention plus its
partial output projection; the host sums the 4 partials per batch and adds bo.

Design (v3):
  - q/k/v transposed on the HOST to [D, S] 16-bit, so stage 0 is pure
    projection matmuls (no PE transposes, no PSUM->SBUF transpose copies).
    V projections run FIRST (they are DVE-coupled and slow regardless);
    q/k run later inside the PE's ramped-clock window.
  - additive bias+mask replaced by multiplicative host-side exp(bias)*mask:
    et = exp(scores/8) * ebias, with scores UNSCALED 16-bit and the
    1/sqrt(hd) folded into the exp's free scale operand. exp reads scores
    PSUM directly on [128, 2x512] paired tiles (amortizes ACT overhead);
    the bias apply is an all-16-bit DVE multiply at 2x rate.
  - softmax denominators come from a ones-column in V through the attn@V
    accumulation; 1/den via ACT ln -> exp(-x) (single table set), GpSimd
    partition_broadcast (reads the tile's partition 0), DVE multiply with
    partition-shifted output for odd heads.
  - everything 16-bit except PSUM (fp32); fp8 was measured and rejected:
    DoubleRow gives no speedup below K=128-on-64 and quantization noise
    does not average in the random-sign attn@V sum (4e-2 rel err).
  - out_p written 16-bit; host sums partials in fp32 and adds bo.
"""
import os
import sys

if "/opt/trn_rl_repo" not in sys.path:
    sys.path.insert(0, "/opt/trn_rl_repo")

from contextlib import ExitStack

import numpy as np

B, S, D, H = 2, 2048, 1024, 16
HD = D // H          # 64
NCORES = 8
HPC = 4              # heads per core
HDC = HPC * HD       # 256 head-dim cols per core
P = 128
ISLAB = 512          # i-columns per score slab
NJT = S // P         # 16 j tiles
NG = NJT // 2        # 8 jt-pairs per (isl, h)
NISLAB = S // ISLAB  # 4 i slabs
NSC = S // ISLAB     # 4 seq chunks in stage 0
LAGP = 3             # ctx trails scores by this many jt-pairs

MMDT = os.environ.get("MMDT", "bf16")     # 16-bit dtype: bf16 | f16
NOEXP = os.environ.get("NOEXP", "0") == "1"  # timing expt: const et, no ACT/DVE
NOMUL = os.environ.get("NOMUL", "0") == "1"  # timing expt: exp only, skip DVE mul
NOACT = os.environ.get("NOACT", "0") == "1"  # timing expt: DVE copy instead of exp

_CACHE = {}


def _build():
    import concourse.bass as bass
    import concourse.mybir as mybir
    import concourse.tile as tile
    from concourse.tile import add_dep_helper
    from concourse import bacc

    f32 = mybir.dt.float32
    f16 = mybir.dt.float16 if MMDT == "f16" else mybir.dt.bfloat16
    EXP = mybir.ActivationFunctionType.Exp
    LN = mybir.ActivationFunctionType.Ln

    # Exp and Ln both live in the 'natural_log_exp_and_others' table set;
    # restricting the registry to it makes insert_act_table_loads emit ONE
    # load instead of thrashing exp<->ln tables at every normalization.
    import concourse.hw_specs as hw_specs
    if not getattr(hw_specs, "_mha_table_patch", False):
        _orig_gat = hw_specs.get_activation_tables

        def _one_table(arch, _orig=_orig_gat):
            t = _orig(arch)
            name = "natural_log_exp_and_others"
            if name not in t:
                return t
            keep = t[name]
            return {
                k: (v if k == name else (v - keep))
                for k, v in t.items()
            }

        hw_specs.get_activation_tables = _one_table
        bacc.get_activation_tables = _one_table
        hw_specs._mha_table_patch = True

    nc = bacc.Bacc(None, target_bir_lowering=False)

    qT = nc.declare_dram_parameter("qT", [D, S], f16, isOutput=False)
    kT = nc.declare_dram_parameter("kT", [D, S], f16, isOutput=False)
    vT = nc.declare_dram_parameter("vT", [D, S], f16, isOutput=False)
    wq = nc.declare_dram_parameter("wq", [D, HDC], f16, isOutput=False)
    wk = nc.declare_dram_parameter("wk", [D, HDC], f16, isOutput=False)
    wv = nc.declare_dram_parameter("wv", [D, HDC], f16, isOutput=False)
    wo = nc.declare_dram_parameter("wo", [HDC, D], f16, isOutput=False)
    bq = nc.declare_dram_parameter("bq", [HDC], f32, isOutput=False)
    bk = nc.declare_dram_parameter("bk", [HDC], f32, isOutput=False)
    bv_rep = nc.declare_dram_parameter("bv_rep", [P, HDC], f32, isOutput=False)
    # eb[h, isl, jtg, p, qq, c] = exp(bias[j, i, h]) * mask[j, i]
    #   with j = (jtg*4 + qq)*128 + p, i = isl*512 + c
    eb = nc.declare_dram_parameter(
        "eb", [HPC, NISLAB, NJT // 4, P, 4, ISLAB], f16, isOutput=False)
    out_p = nc.declare_dram_parameter("out_p", [S, D], f16, isOutput=True)

    with tile.TileContext(nc) as tc, ExitStack() as big:
        consts = big.enter_context(tc.tile_pool(name="consts", bufs=1))
        persist = big.enter_context(tc.tile_pool(name="persist", bufs=1))

        ones_col = consts.tile([P, 1], f16)
        nc.vector.memset(ones_col, 1.0)
        # warm the exp/ln act table while stage 0 runs
        warm = consts.tile([P, 1], f32)
        nc.scalar.activation(warm, ones_col, EXP)

        bqv = consts.tile([P, 2], f32)
        bkv = consts.tile([P, 2], f32)
        nc.sync.dma_start(bkv, bk[:].rearrange("(o p) -> p o", p=P))
        nc.sync.dma_start(bqv, bq[:].rearrange("(o p) -> p o", p=P))

        wq_sb = consts.tile([P, 8, HDC], f16)
        wk_sb = consts.tile([P, 8, HDC], f16)
        wv_sb = consts.tile([P, 8, HDC], f16)
        wo_sb = consts.tile([P, 2, D], f16)

        wmap = {"k": (wk_sb, wk), "v": (wv_sb, wv), "q": (wq_sb, wq)}

        def load_weight(name):
            # staggered so the first user never waits on a lumped completion
            # semaphore covering later weight loads
            sb, dr = wmap[name]
            nc.sync.dma_start(sb, dr[:].rearrange("(dk p) m -> p dk m", p=P))

        def load_wo():
            nc.sync.dma_start(wo_sb, wo[:].rearrange("(kt p) n -> p kt n", p=P))

        qt_f = persist.tile([P, 2, S], f16)   # [hd%64+64*(h%2), h//2, s]
        kt_f = persist.tile([P, 2, S], f16)
        v_full = persist.tile([P, NJT, HPC, HD + 1], f16)  # [j%128, jt, h, hd|1]
        ctxT = persist.tile([P, 2, S], f16)   # [hd%128, hd//128, i]

        # ones column of V (softmax denominator trick)
        nc.vector.tensor_copy(
            v_full[:, :, :, HD:HD + 1],
            ones_col[:, None, None, :].to_broadcast((P, NJT, HPC, 1)))

        # ---------------- Stage 0: projections --------------------------
        # V first: its PSUM->SBUF DVE adds throttle the PE clock anyway, so
        # it absorbs the clock ramp-up; q/k then stream in the fast window.
        s0 = big.enter_context(tc.tile_pool(name="s0", bufs=3))
        with ExitStack() as st0:
            pps = st0.enter_context(tc.tile_pool(name="pps", bufs=2, space="PSUM"))
            vps = st0.enter_context(tc.tile_pool(name="vps", bufs=4, space="PSUM"))

            first = True
            for x_dram, which in ((kT, "k"), (vT, "v"), (qT, "q")):
                for sc in range(NSC):
                    sl = slice(sc * ISLAB, (sc + 1) * ISLAB)
                    xn = s0.tile([P, 8, ISLAB], f16, tag="xn")
                    if first:
                        # split the very first input and weight loads into
                        # interleaved dk-chunks so the first accumulation
                        # chain starts as soon as chunk 0 of each lands
                        wk_r = wk[:].rearrange("(dk p) m -> p dk m", p=P)
                        xn_r = x_dram[:, sl].rearrange("(dk p) s -> p dk s",
                                                       p=P)
                        for dk4 in range(4):
                            c = slice(2 * dk4, 2 * dk4 + 2)
                            nc.sync.dma_start(wk_sb[:, c, :], wk_r[:, c, :])
                            nc.sync.dma_start(xn[:, c, :], xn_r[:, c, :])
                        first = False
                    else:
                        nc.sync.dma_start(
                            xn, x_dram[:, sl].rearrange("(dk p) s -> p dk s",
                                                        p=P))
                    if which == "k" and sc == 1:
                        load_weight("v")
                    if which == "k" and sc == 2:
                        load_weight("q")
                    if which == "v" and sc == 0:
                        load_wo()
                    if which == "v":
                        for st in range(4):
                            vp = vps.tile([P, HDC], f32, tag="vp")
                            for dk in range(8):
                                nc.tensor.matmul(
                                    vp, xn[:, dk, st * P:(st + 1) * P],
                                    wv_sb[:, dk, :],
                                    start=(dk == 0), stop=(dk == 7),
                                )
                            jt = sc * 4 + st
                            # bv is folded into bo host-side (softmax weights
                            # sum to 1, so V+bv shifts ctx by the constant bv)
                            if st % 2 == 0:
                                nc.vector.tensor_copy(
                                    v_full[:, jt, :, :HD],
                                    vp.rearrange("p (h d) -> p h d", h=HPC))
                            else:
                                nc.scalar.copy(
                                    v_full[:, jt, :, :HD],
                                    vp.rearrange("p (h d) -> p h d", h=HPC))
                    else:
                        dst = qt_f if which == "q" else kt_f
                        w_sb = wq_sb if which == "q" else wk_sb
                        bvec = bqv if which == "q" else bkv
                        for mt in range(2):
                            pp = pps.tile([P, ISLAB], f32, tag="pp")
                            for dk in range(8):
                                nc.tensor.matmul(
                                    pp, w_sb[:, dk, mt * P:(mt + 1) * P],
                                    xn[:, dk, :],
                                    start=(dk == 0), stop=(dk == 7),
                                )
                            nc.vector.tensor_scalar_add(
                                dst[:, mt, sl], pp, bvec[:, mt:mt + 1])

        # ---------------- Stage 1: attention ------------------------------
        sbias = big.enter_context(tc.tile_pool(name="sbias", bufs=5))
        ses = big.enter_context(tc.tile_pool(name="ses", bufs=3))
        sexp = big.enter_context(tc.tile_pool(name="sexp", bufs=6))
        snrm = big.enter_context(tc.tile_pool(name="snrm", bufs=2))
        so = big.enter_context(tc.tile_pool(name="so", bufs=3))
        if NOEXP:
            et_const = consts.tile([P, 2, ISLAB], f16)
            nc.vector.memset(et_const, 0.001)
        if NOMUL:
            pass
        with ExitStack() as st1:
            sps = st1.enter_context(tc.tile_pool(name="sps", bufs=3, space="PSUM"))
            cps = st1.enter_context(tc.tile_pool(name="cps", bufs=2, space="PSUM"))

            def make_norm_steps(cp, hp, ho, isl):
                # Deferred softmax normalization for one finished slab-head,
                # emitted as discrete steps interleaved into the NEXT head.
                state = {}

                def s_ln():
                    # partition-shifted ACT output (64 -> 0): hw
                    # partition_broadcast always reads the tile's partition 0
                    state["ln"] = snrm.tile([1, ISLAB], f32, tag="ln",
                                            name="ln")
                    nc.scalar.activation(state["ln"], cp[HD:HD + 1, :], LN)

                def s_recip():
                    state["rt"] = snrm.tile([1, ISLAB], f32, tag="rt",
                                            name="rt")
                    nc.scalar.activation(state["rt"], state["ln"], EXP,
                                         scale=-1.0)

                def s_bcast():
                    state["bc"] = snrm.tile([P, ISLAB], f32, tag="bc",
                                            name="bc")
                    nc.gpsimd.partition_broadcast(state["bc"], state["rt"])

                def s_mul():
                    nc.vector.tensor_mul(
                        ctxT[hp:hp + 64, ho, isl * ISLAB:(isl + 1) * ISLAB],
                        cp[:HD, :], state["bc"][0:64, :],
                    )

                return [s_ln, s_recip, s_bcast, s_mul]

            pending = []
            NORM_AT = {1: 0, 2: 1, 3: 2, 5: 3}
            pending_out = []

            # eb tile prefetcher: stays ~2 jt-pair-groups ahead, across
            # head boundaries, so the et multiply never waits on HBM.
            eb_order = [(i_, h_, g_) for i_ in range(NISLAB)
                        for h_ in range(HPC) for g_ in range(NJT // 4)]
            eb_tiles = {}
            eb_ptr = [0]

            def pump_eb(n=1):
                for _ in range(n):
                    if eb_ptr[0] >= len(eb_order):
                        return
                    i_, h_, g_ = eb_order[eb_ptr[0]]
                    t = sbias.tile([P, 4, ISLAB], f16, tag="ebt")
                    nc.sync.dma_start(t, eb[h_, i_, g_])
                    eb_tiles[(i_, h_, g_)] = t
                    eb_ptr[0] += 1

            pump_eb(2)

            def make_outproj(isl):
                # output projection for one finished i-slab, split into 8
                # steps so each briefly borrows one sp-ring slot instead of
                # bursting through the ring.
                def step(it, nt):
                    def run():
                        opt_ = sps.tile([P, 2, ISLAB], f32, tag="sp",
                                        name="op")
                        op = opt_[:, 0, :]
                        for kt_ in range(2):
                            nc.tensor.matmul(
                                op, ctxT[:, kt_, it * P:(it + 1) * P],
                                wo_sb[:, kt_, nt * ISLAB:(nt + 1) * ISLAB],
                                start=(kt_ == 0), stop=(kt_ == 1),
                            )
                        ot = so.tile([P, ISLAB], f16, tag="ot", name="ot")
                        nc.vector.tensor_copy(ot, op)
                        nc.sync.dma_start(
                            out_p[it * P:(it + 1) * P,
                                  nt * ISLAB:(nt + 1) * ISLAB], ot)
                    return run
                return [step(it, nt) for it in range(isl * 4, isl * 4 + 4)
                        for nt in range(2)]

            for isl in range(NISLAB):
                isl_sl = slice(isl * ISLAB, (isl + 1) * ISLAB)
                for h in range(HPC):
                    hp = (h % 2) * 64   # base partition of this head
                    ho = h // 2         # outer index
                    qt_h = qt_f[hp:hp + HD, h // 2, isl_sl]
                    cp = cps.tile([HD + 1, ISLAB], f32, tag="cp")
                    ets = [None] * NG
                    sc_last = [None] * NG
                    ebt_cur = None
                    for g in range(NG + LAGP):
                        if g in NORM_AT and pending:
                            pending[NORM_AT[g]]()
                        if h >= 2 and g < 6 and pending_out:
                            pending_out.pop(0)()
                        if g < NG:
                            if g % 2 == 0:
                                pump_eb(1)
                                ebt_cur = eb_tiles.pop((isl, h, g // 2))
                            sp = sps.tile([P, 2, ISLAB], f32, tag="sp")
                            for q in range(2):
                                jt = 2 * g + q
                                smm = nc.tensor.matmul(
                                    sp[:, q, :],
                                    kt_f[hp:hp + HD, h // 2,
                                         jt * P:(jt + 1) * P],
                                    qt_h, start=True, stop=True)
                            sc_last[g] = smm
                            if NOEXP:
                                ets[g] = et_const
                            elif NOMUL:
                                es = ses.tile([P, 2, ISLAB], f16, tag="es")
                                nc.scalar.activation(es, sp, EXP, scale=0.125)
                                ets[g] = es
                            elif NOACT:
                                es = ses.tile([P, 2, ISLAB], f16, tag="es")
                                nc.vector.tensor_copy(es, sp)
                                et = sexp.tile([P, 2, ISLAB], f16, tag="et")
                                nc.vector.tensor_mul(
                                    et, es,
                                    ebt_cur[:, (g % 2) * 2:(g % 2) * 2 + 2, :])
                                ets[g] = et
                            else:
                                es = ses.tile([P, 2, ISLAB], f16, tag="es")
                                nc.scalar.activation(es, sp, EXP, scale=0.125)
                                et = sexp.tile([P, 2, ISLAB], f16, tag="et")
                                nc.vector.tensor_mul(
                                    et, es,
                                    ebt_cur[:, (g % 2) * 2:(g % 2) * 2 + 2, :])
                                ets[g] = et
                        if g >= LAGP:
                            g2 = g - LAGP
                            for q in range(2):
                                j2 = 2 * g2 + q
                                cmm = nc.tensor.matmul(
                                    cp, v_full[:, j2, h, :], ets[g2][:, q, :],
                                    start=(j2 == 0), stop=(j2 == NJT - 1),
                                )
                            if g < NG:
                                # keep the software-pipeline skew in the PE
                                # stream: ctx(g2) goes AFTER scores(g)
                                add_dep_helper(
                                    sc_last[g].ins, cmm.ins, sync=False,
                                    reason="preserve scores/ctx LAG skew")
                    pending = make_norm_steps(cp, hp, ho, isl)
                pending_out.extend(make_outproj(isl))
            for step in pending:
                step()
            for run in pending_out:
                run()

    nc.compile()
    return nc


def _get_nc():
    if "nc" not in _CACHE:
        _CACHE["nc"] = _build()
    return _CACHE["nc"]


def _prep_inputs(query, key, value, mask, relative_pos_bias,
                 Wq, bq, Wk, bk, Wv, bv, Wo, bo):
    import ml_dtypes
    f32 = np.float32
    f16 = np.float16 if MMDT == "f16" else ml_dtypes.bfloat16
    query = np.asarray(query, f32)
    key = np.asarray(key, f32)
    value = np.asarray(value, f32)
    # eb[h, j, i] = exp(bias[j, i, h]); mask folded per batch below
    eb_h = np.exp(np.asarray(relative_pos_bias, f32)).transpose(2, 0, 1)
    mask_ji = (np.asarray(mask)[:, 0] != 0).transpose(0, 2, 1).astype(f32)

    Wq_ = np.asarray(Wq, f32)
    Wk_ = np.asarray(Wk, f32)
    Wv_ = np.asarray(Wv, f32)
    Wo_ = np.asarray(Wo, f32)
    bq_ = np.asarray(bq, f32)
    bk_ = np.asarray(bk, f32)
    bv_ = np.asarray(bv, f32)

    xT = {}
    for b in range(B):
        xT[("q", b)] = np.ascontiguousarray(query[b].T).astype(f16)
        xT[("k", b)] = np.ascontiguousarray(key[b].T).astype(f16)
        xT[("v", b)] = np.ascontiguousarray(value[b].T).astype(f16)

    in_maps = []
    for c in range(NCORES):
        b = c // 4
        h0 = (c % 4) * HPC
        cols = slice(h0 * HD, (h0 + HPC) * HD)
        ebc = eb_h[h0:h0 + HPC] * mask_ji[b][None]      # [4, S(j), S(i)]
        ebc = ebc.reshape(HPC, NJT // 4, 4, P, NISLAB, ISLAB)
        ebc = np.ascontiguousarray(
            ebc.transpose(0, 4, 1, 3, 2, 5)).astype(f16)
        in_maps.append({
            "qT": xT[("q", b)],
            "kT": xT[("k", b)],
            "vT": xT[("v", b)],
            "wq": np.ascontiguousarray(Wq_[:, cols]).astype(f16),
            "wk": np.ascontiguousarray(Wk_[:, cols]).astype(f16),
            "wv": np.ascontiguousarray(Wv_[:, cols]).astype(f16),
            "wo": np.ascontiguousarray(Wo_[cols, :]).astype(f16),
            "bq": np.ascontiguousarray(bq_[cols]),
            "bk": np.ascontiguousarray(bk_[cols]),
            "bv_rep": np.ascontiguousarray(
                np.broadcast_to(bv_[cols], (P, HDC))),
            "eb": ebc,
        })
    return in_maps


def run_sharded(run_kwargs=None, **inputs):
    """Build + run on 8 cores; returns (output, BassKernelResults)."""
    from concourse.bass_utils import run_bass_kernel_spmd

    nc = _get_nc()
    in_maps = _prep_inputs(**inputs)
    res = run_bass_kernel_spmd(nc, in_maps, list(range(NCORES)),
                               **(run_kwargs or {}))
    bo = np.asarray(inputs["bo"], np.float32)
    bv_ = np.asarray(inputs["bv"], np.float32)
    Wo_ = np.asarray(inputs["Wo"], np.float32)
    out = np.zeros((B, S, D), np.float32)
    for c in range(NCORES):
        out[c // 4] += res.results[c]["out_p"].astype(np.float32)
    out += (bo + bv_ @ Wo_)[None, None, :]
    return out, res


def kernel(**inputs):
    out, _ = run_sharded(**inputs)
    return out


# revision 38
# speedup vs baseline: 1.1182x; 1.1182x over previous
"""Multi-head attention TRN2 kernel, sharded over 8 NeuronCores.

Sharding: (batch, head-group) - core c handles batch c//4 and heads
(c%4)*4 .. (c%4)*4+3. Each core computes its 4 heads' attention plus its
partial output projection; the host sums the 4 partials per batch and adds bo.

Design (v3):
  - q/k/v transposed on the HOST to [D, S] 16-bit, so stage 0 is pure
    projection matmuls (no PE transposes, no PSUM->SBUF transpose copies).
    V projections run FIRST (they are DVE-coupled and slow regardless);
    q/k run later inside the PE's ramped-clock window.
  - additive bias+mask replaced by multiplicative host-side exp(bias)*mask:
    et = exp(scores/8) * ebias, with scores UNSCALED 16-bit and the
    1/sqrt(hd) folded into the exp's free scale operand. exp reads scores
    PSUM directly on [128, 2x512] paired tiles (amortizes ACT overhead);
    the bias apply is an all-16-bit DVE multiply at 2x rate.
  - softmax denominators come from a ones-column in V through the attn@V
    accumulation; 1/den via ACT ln -> exp(-x) (single table set), GpSimd
    partition_broadcast (reads the tile's partition 0), DVE multiply with
    partition-shifted output for odd heads.
  - everything 16-bit except PSUM (fp32); fp8 was measured and rejected:
    DoubleRow gives no speedup below K=128-on-64 and quantization noise
    does not average in the random-sign attn@V sum (4e-2 rel err).
  - out_p written 16-bit; host sums partials in fp32 and adds bo.
"""
import os
import sys

if "/opt/trn_rl_repo" not in sys.path:
    sys.path.insert(0, "/opt/trn_rl_repo")

from contextlib import ExitStack

import numpy as np

B, S, D, H = 2, 2048, 1024, 16
HD = D // H          # 64
NCORES = 8
HPC = 4              # heads per core
HDC = HPC * HD       # 256 head-dim cols per core
P = 128
ISLAB = 512          # i-columns per score slab
NJT = S // P         # 16 j tiles
NG = NJT // 2        # 8 jt-pairs per (isl, h)
NISLAB = S // ISLAB  # 4 i slabs
NSC = S // ISLAB     # 4 seq chunks in stage 0
LAGP = 3             # ctx trails scores by this many jt-pairs

MMDT = os.environ.get("MMDT", "bf16")     # 16-bit dtype: bf16 | f16
NOEXP = os.environ.get("NOEXP", "0") == "1"  # timing expt: const et, no ACT/DVE
NOMUL = os.environ.get("NOMUL", "0") == "1"  # timing expt: exp only, skip DVE mul
NOACT = os.environ.get("NOACT", "0") == "1"  # timing expt: DVE copy instead of exp

_CACHE = {}


def _build():
    import concourse.bass as bass
    import concourse.mybir as mybir
    import concourse.tile as tile
    from concourse.tile import add_dep_helper
    from concourse import bacc

    f32 = mybir.dt.float32
    f16 = mybir.dt.float16 if MMDT == "f16" else mybir.dt.bfloat16
    EXP = mybir.ActivationFunctionType.Exp
    LN = mybir.ActivationFunctionType.Ln

    # Exp and Ln both live in the 'natural_log_exp_and_others' table set;
    # restricting the registry to it makes insert_act_table_loads emit ONE
    # load instead of thrashing exp<->ln tables at every normalization.
    import concourse.hw_specs as hw_specs
    if not getattr(hw_specs, "_mha_table_patch", False):
        _orig_gat = hw_specs.get_activation_tables

        def _one_table(arch, _orig=_orig_gat):
            t = _orig(arch)
            name = "natural_log_exp_and_others"
            if name not in t:
                return t
            keep = t[name]
            return {
                k: (v if k == name else (v - keep))
                for k, v in t.items()
            }

        hw_specs.get_activation_tables = _one_table
        bacc.get_activation_tables = _one_table
        hw_specs._mha_table_patch = True

    nc = bacc.Bacc(None, target_bir_lowering=False)

    qT = nc.declare_dram_parameter("qT", [D, S], f16, isOutput=False)
    kT = nc.declare_dram_parameter("kT", [D, S], f16, isOutput=False)
    vT = nc.declare_dram_parameter("vT", [D, S], f16, isOutput=False)
    wq = nc.declare_dram_parameter("wq", [D, HDC], f16, isOutput=False)
    wk = nc.declare_dram_parameter("wk", [D, HDC], f16, isOutput=False)
    wv = nc.declare_dram_parameter("wv", [D, HDC], f16, isOutput=False)
    wo = nc.declare_dram_parameter("wo", [HDC, D], f16, isOutput=False)
    bq = nc.declare_dram_parameter("bq", [HDC], f32, isOutput=False)
    bk = nc.declare_dram_parameter("bk", [HDC], f32, isOutput=False)
    bv_rep = nc.declare_dram_parameter("bv_rep", [P, HDC], f32, isOutput=False)
    # eb[h, isl, jtg, p, qq, c] = exp(bias[j, i, h]) * mask[j, i]
    #   with j = (jtg*4 + qq)*128 + p, i = isl*512 + c
    eb = nc.declare_dram_parameter(
        "eb", [HPC, NISLAB, NJT // 4, P, 4, ISLAB], f16, isOutput=False)
    out_p = nc.declare_dram_parameter("out_p", [S, D], f16, isOutput=True)

    with tile.TileContext(nc) as tc, ExitStack() as big:
        consts = big.enter_context(tc.tile_pool(name="consts", bufs=1))
        persist = big.enter_context(tc.tile_pool(name="persist", bufs=1))

        ones_col = consts.tile([P, 1], f16)
        nc.vector.memset(ones_col, 1.0)
        # warm the exp/ln act table while stage 0 runs
        warm = consts.tile([P, 1], f32)
        nc.scalar.activation(warm, ones_col, EXP)

        bqv = consts.tile([P, 2], f32)
        bkv = consts.tile([P, 2], f32)
        nc.sync.dma_start(bkv, bk[:].rearrange("(o p) -> p o", p=P))
        nc.sync.dma_start(bqv, bq[:].rearrange("(o p) -> p o", p=P))

        wq_sb = consts.tile([P, 8, HDC], f16)
        wk_sb = consts.tile([P, 8, HDC], f16)
        wv_sb = consts.tile([P, 8, HDC], f16)
        wo_sb = consts.tile([P, 2, D], f16)

        wmap = {"k": (wk_sb, wk), "v": (wv_sb, wv), "q": (wq_sb, wq)}

        def load_weight(name):
            # staggered so the first user never waits on a lumped completion
            # semaphore covering later weight loads
            sb, dr = wmap[name]
            nc.sync.dma_start(sb, dr[:].rearrange("(dk p) m -> p dk m", p=P))

        def load_wo():
            nc.sync.dma_start(wo_sb, wo[:].rearrange("(kt p) n -> p kt n", p=P))

        qt_f = persist.tile([P, 2, S], f16)   # [hd%64+64*(h%2), h//2, s]
        kt_f = persist.tile([P, 2, S], f16)
        v_full = persist.tile([P, NJT, HPC, HD + 1], f16)  # [j%128, jt, h, hd|1]
        ctxT = persist.tile([P, 2, S], f16)   # [hd%128, hd//128, i]

        # ones column of V (softmax denominator trick)
        nc.vector.tensor_copy(
            v_full[:, :, :, HD:HD + 1],
            ones_col[:, None, None, :].to_broadcast((P, NJT, HPC, 1)))

        # ---------------- Stage 0: projections --------------------------
        # V first: its PSUM->SBUF DVE adds throttle the PE clock anyway, so
        # it absorbs the clock ramp-up; q/k then stream in the fast window.
        s0 = big.enter_context(tc.tile_pool(name="s0", bufs=3))
        with ExitStack() as st0:
            pps = st0.enter_context(tc.tile_pool(name="pps", bufs=2, space="PSUM"))
            vps = st0.enter_context(tc.tile_pool(name="vps", bufs=4, space="PSUM"))

            first = True
            for x_dram, which in ((kT, "k"), (vT, "v"), (qT, "q")):
                for sc in range(NSC):
                    sl = slice(sc * ISLAB, (sc + 1) * ISLAB)
                    xn = s0.tile([P, 8, ISLAB], f16, tag="xn")
                    if first:
                        # split the very first input and weight loads into
                        # interleaved dk-chunks so the first accumulation
                        # chain starts as soon as chunk 0 of each lands
                        wk_r = wk[:].rearrange("(dk p) m -> p dk m", p=P)
                        xn_r = x_dram[:, sl].rearrange("(dk p) s -> p dk s",
                                                       p=P)
                        for dk4 in range(4):
                            c = slice(2 * dk4, 2 * dk4 + 2)
                            nc.sync.dma_start(wk_sb[:, c, :], wk_r[:, c, :])
                            nc.sync.dma_start(xn[:, c, :], xn_r[:, c, :])
                        first = False
                    else:
                        nc.sync.dma_start(
                            xn, x_dram[:, sl].rearrange("(dk p) s -> p dk s",
                                                        p=P))
                    if which == "k" and sc == 1:
                        load_weight("v")
                    if which == "k" and sc == 2:
                        load_weight("q")
                    if which == "v" and sc == 0:
                        load_wo()
                    if which == "v":
                        for st in range(4):
                            vp = vps.tile([P, HDC], f32, tag="vp")
                            for dk in range(8):
                                nc.tensor.matmul(
                                    vp, xn[:, dk, st * P:(st + 1) * P],
                                    wv_sb[:, dk, :],
                                    start=(dk == 0), stop=(dk == 7),
                                )
                            jt = sc * 4 + st
                            # bv is folded into bo host-side (softmax weights
                            # sum to 1, so V+bv shifts ctx by the constant bv)
                            if st % 2 == 0:
                                nc.vector.tensor_copy(
                                    v_full[:, jt, :, :HD],
                                    vp.rearrange("p (h d) -> p h d", h=HPC))
                            else:
                                nc.scalar.copy(
                                    v_full[:, jt, :, :HD],
                                    vp.rearrange("p (h d) -> p h d", h=HPC))
                    else:
                        dst = qt_f if which == "q" else kt_f
                        w_sb = wq_sb if which == "q" else wk_sb
                        bvec = bqv if which == "q" else bkv
                        for mt in range(2):
                            pp = pps.tile([P, ISLAB], f32, tag="pp")
                            for dk in range(8):
                                nc.tensor.matmul(
                                    pp, w_sb[:, dk, mt * P:(mt + 1) * P],
                                    xn[:, dk, :],
                                    start=(dk == 0), stop=(dk == 7),
                                )
                            nc.vector.tensor_scalar_add(
                                dst[:, mt, sl], pp, bvec[:, mt:mt + 1])

        # ---------------- Stage 1: attention ------------------------------
        sbias = big.enter_context(tc.tile_pool(name="sbias", bufs=5))
        ses = big.enter_context(tc.tile_pool(name="ses", bufs=3))
        sexp = big.enter_context(tc.tile_pool(name="sexp", bufs=6))
        snrm = big.enter_context(tc.tile_pool(name="snrm", bufs=2))
        so = big.enter_context(tc.tile_pool(name="so", bufs=3))
        if NOEXP:
            et_const = consts.tile([P, 2, ISLAB], f16)
            nc.vector.memset(et_const, 0.001)
        if NOMUL:
            pass
        with ExitStack() as st1:
            sps = st1.enter_context(tc.tile_pool(name="sps", bufs=3, space="PSUM"))
            cps = st1.enter_context(tc.tile_pool(name="cps", bufs=2, space="PSUM"))

            def make_norm_steps(cp, hp, ho, isl):
                # Deferred softmax normalization for one finished slab-head,
                # emitted as discrete steps interleaved into the NEXT head.
                state = {}

                def s_ln():
                    # partition-shifted ACT output (64 -> 0): hw
                    # partition_broadcast always reads the tile's partition 0
                    state["ln"] = snrm.tile([1, ISLAB], f32, tag="ln",
                                            name="ln")
                    nc.scalar.activation(state["ln"], cp[HD:HD + 1, :], LN)

                def s_recip():
                    state["rt"] = snrm.tile([1, ISLAB], f32, tag="rt",
                                            name="rt")
                    nc.scalar.activation(state["rt"], state["ln"], EXP,
                                         scale=-1.0)

                def s_bcast():
                    state["bc"] = snrm.tile([P, ISLAB], f32, tag="bc",
                                            name="bc")
                    nc.gpsimd.partition_broadcast(state["bc"], state["rt"])

                def s_mul():
                    nc.vector.tensor_mul(
                        ctxT[hp:hp + 64, ho, isl * ISLAB:(isl + 1) * ISLAB],
                        cp[:HD, :], state["bc"][0:64, :],
                    )

                return [s_ln, s_recip, s_bcast, s_mul]

            pending = []
            NORM_AT = {1: 0, 2: 1, 3: 2, 5: 3}
            pending_out = []

            # eb tile prefetcher: stays ~2 jt-pair-groups ahead, across
            # head boundaries, so the et multiply never waits on HBM.
            eb_order = [(i_, h_, g_) for i_ in range(NISLAB)
                        for h_ in range(HPC) for g_ in range(NJT // 4)]
            eb_tiles = {}
            eb_ptr = [0]

            def pump_eb(n=1):
                for _ in range(n):
                    if eb_ptr[0] >= len(eb_order):
                        return
                    i_, h_, g_ = eb_order[eb_ptr[0]]
                    t = sbias.tile([P, 4, ISLAB], f16, tag="ebt")
                    nc.sync.dma_start(t, eb[h_, i_, g_])
                    eb_tiles[(i_, h_, g_)] = t
                    eb_ptr[0] += 1

            pump_eb(2)

            def make_outproj(isl):
                # output projection for one finished i-slab, split into 8
                # steps so each briefly borrows one sp-ring slot instead of
                # bursting through the ring.
                def step(it, nt):
                    def run():
                        opt_ = sps.tile([P, 2, ISLAB], f32, tag="sp",
                                        name="op")
                        op = opt_[:, 0, :]
                        for kt_ in range(2):
                            nc.tensor.matmul(
                                op, ctxT[:, kt_, it * P:(it + 1) * P],
                                wo_sb[:, kt_, nt * ISLAB:(nt + 1) * ISLAB],
                                start=(kt_ == 0), stop=(kt_ == 1),
                            )
                        ot = so.tile([P, ISLAB], f16, tag="ot", name="ot")
                        nc.vector.tensor_copy(ot, op)
                        nc.sync.dma_start(
                            out_p[it * P:(it + 1) * P,
                                  nt * ISLAB:(nt + 1) * ISLAB], ot)
                    return run
                return [step(it, nt) for it in range(isl * 4, isl * 4 + 4)
                        for nt in range(2)]

            for isl in range(NISLAB):
                isl_sl = slice(isl * ISLAB, (isl + 1) * ISLAB)
                for h in range(HPC):
                    hp = (h % 2) * 64   # base partition of this head
                    ho = h // 2         # outer index
                    qt_h = qt_f[hp:hp + HD, h // 2, isl_sl]
                    cp = cps.tile([HD + 1, ISLAB], f32, tag="cp")
                    ets = [None] * NG
                    sc_last = [None] * NG
                    ebt_cur = None
                    for g in range(NG + LAGP):
                        if g in NORM_AT and pending:
                            pending[NORM_AT[g]]()
                        if h >= 2 and g < 6 and pending_out:
                            pending_out.pop(0)()
                        if g < NG:
                            if g % 2 == 0:
                                pump_eb(1)
                                ebt_cur = eb_tiles.pop((isl, h, g // 2))
                            sp = sps.tile([P, 2, ISLAB], f32, tag="sp")
                            for q in range(2):
                                jt = 2 * g + q
                                smm = nc.tensor.matmul(
                                    sp[:, q, :],
                                    kt_f[hp:hp + HD, h // 2,
                                         jt * P:(jt + 1) * P],
                                    qt_h, start=True, stop=True)
                            sc_last[g] = smm
                            if NOEXP:
                                ets[g] = et_const
                            elif NOMUL:
                                es = ses.tile([P, 2, ISLAB], f16, tag="es")
                                nc.scalar.activation(es, sp, EXP, scale=0.125)
                                ets[g] = es
                            elif NOACT:
                                es = ses.tile([P, 2, ISLAB], f16, tag="es")
                                nc.vector.tensor_copy(es, sp)
                                et = sexp.tile([P, 2, ISLAB], f16, tag="et")
                                nc.vector.tensor_mul(
                                    et, es,
                                    ebt_cur[:, (g % 2) * 2:(g % 2) * 2 + 2, :])
                                ets[g] = et
                            else:
                                es = ses.tile([P, 2, ISLAB], f16, tag="es")
                                nc.scalar.activation(es, sp, EXP, scale=0.125)
                                et = sexp.tile([P, 2, ISLAB], f16, tag="et")
                                nc.vector.tensor_mul(
                                    et, es,
                                    ebt_cur[:, (g % 2) * 2:(g % 2) * 2 + 2, :])
                                ets[g] = et
                        if g >= LAGP:
                            g2 = g - LAGP
                            for q in range(2):
                                j2 = 2 * g2 + q
                                cmm = nc.tensor.matmul(
                                    cp, v_full[:, j2, h, :], ets[g2][:, q, :],
                                    start=(j2 == 0), stop=(j2 == NJT - 1),
                                )
                            if g < NG:
                                # keep the software-pipeline skew in the PE
                                # stream: ctx(g2) goes AFTER scores(g)
                                add_dep_helper(
                                    sc_last[g].ins, cmm.ins, sync=False,
                                    reason="preserve scores/ctx LAG skew")
                    pending = make_norm_steps(cp, hp, ho, isl)
                pending_out.extend(make_outproj(isl))
            for step in pending:
                step()
            for run in pending_out:
                run()

    nc.compile()
    return nc


def _get_nc():
    if "nc" not in _CACHE:
        _CACHE["nc"] = _build()
    return _CACHE["nc"]


def _prep_inputs(query, key, value, mask, relative_pos_bias,
                 Wq, bq, Wk, bk, Wv, bv, Wo, bo):
    import ml_dtypes
    f32 = np.float32
    f16 = np.float16 if MMDT == "f16" else ml_dtypes.bfloat16
    query = np.asarray(query, f32)
    key = np.asarray(key, f32)
    value = np.asarray(value, f32)
    # eb[h, j, i] = exp(bias[j, i, h]); mask folded per batch below
    eb_h = np.exp(np.asarray(relative_pos_bias, f32)).transpose(2, 0, 1)
    mask_ji = (np.asarray(mask)[:, 0] != 0).transpose(0, 2, 1).astype(f32)

    Wq_ = np.asarray(Wq, f32)
    Wk_ = np.asarray(Wk, f32)
    Wv_ = np.asarray(Wv, f32)
    Wo_ = np.asarray(Wo, f32)
    bq_ = np.asarray(bq, f32)
    bk_ = np.asarray(bk, f32)
    bv_ = np.asarray(bv, f32)

    xT = {}
    for b in range(B):
        xT[("q", b)] = np.ascontiguousarray(query[b].T).astype(f16)
        xT[("k", b)] = np.ascontiguousarray(key[b].T).astype(f16)
        xT[("v", b)] = np.ascontiguousarray(value[b].T).astype(f16)

    in_maps = []
    for c in range(NCORES):
        b = c // 4
        h0 = (c % 4) * HPC
        cols = slice(h0 * HD, (h0 + HPC) * HD)
        ebc = eb_h[h0:h0 + HPC] * mask_ji[b][None]      # [4, S(j), S(i)]
        ebc = ebc.reshape(HPC, NJT // 4, 4, P, NISLAB, ISLAB)
        ebc = np.ascontiguousarray(
            ebc.transpose(0, 4, 1, 3, 2, 5)).astype(f16)
        in_maps.append({
            "qT": xT[("q", b)],
            "kT": xT[("k", b)],
            "vT": xT[("v", b)],
            "wq": np.ascontiguousarray(Wq_[:, cols]).astype(f16),
            "wk": np.ascontiguousarray(Wk_[:, cols]).astype(f16),
            "wv": np.ascontiguousarray(Wv_[:, cols]).astype(f16),
            "wo": np.ascontiguousarray(Wo_[cols, :]).astype(f16),
            "bq": np.ascontiguousarray(bq_[cols]),
            "bk": np.ascontiguousarray(bk_[cols]),
            "bv_rep": np.ascontiguousarray(
                np.broadcast_to(bv_[cols], (P, HDC))),
            "eb": ebc,
        })
    return in_maps


def run_sharded(run_kwargs=None, **inputs):
    """Build + run on 8 cores; returns (output, BassKernelResults)."""
    from concourse.bass_utils import run_bass_kernel_spmd

    nc = _get_nc()
    in_maps = _prep_inputs(**inputs)
    res = run_bass_kernel_spmd(nc, in_maps, list(range(NCORES)),
                               **(run_kwargs or {}))
    bo = np.asarray(inputs["bo"], np.float32)
    bv_ = np.asarray(inputs["bv"], np.float32)
    Wo_ = np.asarray(inputs["Wo"], np.float32)
    out = np.zeros((B, S, D), np.float32)
    for c in range(NCORES):
        out[c // 4] += res.results[c]["out_p"].astype(np.float32)
    out += (bo + bv_ @ Wo_)[None, None, :]
    return out, res


def kernel(**inputs):
    out, _ = run_sharded(**inputs)
    return out
